# revision 32
# baseline (speedup 1.0000x reference)
"""Bass/Trainium2 kernel for nn_DCDicl (DSBlock forward).

Algorithm: instead of the O(K^2 * R) unfold-Gram (baseline), compute the
all-pairs shift correlation corr[j,i,u,v] = sum_{h,w} x[j,h,w] *
xpad[i,h+u-4,w+v-4] (8x fewer FLOPs — the Gram is a Toeplitz gather of
corr), plus the U^T y rows for P folded into the same matmuls.

Device (8 cores = 4 samples x 2 w-halves, bf16 in / fp32 psum):
  out[m, (u,i,v)] = sum_{h, w in half} XY[m,h,w] * xpad[i, h+u, w+v]
with contraction over h (96 partitions) and PSUM accumulation over w.
Host: sum halves, gather Q via a sliding-window view, fp32 Cholesky solve.
"""

import sys
import time

import numpy as np

if "/opt/trn_rl_repo" not in sys.path:
    sys.path.append("/opt/trn_rl_repo")

N, C_IN, C_OUT, H, W, DS = 4, 64, 4, 96, 96, 5
K = C_IN * DS * DS          # 1600
NU = 2 * DS - 1             # 9 shifts per axis
M = C_IN + C_OUT            # 68 lhs rows (64 x-channels + 4 y-channels)
WH = W // 2                 # 48 w-columns per core (contraction half)
WV = WH + NU - 1            # 56 w-columns of padded image needed per core
HP = H + 2 * (DS - 1)       # 104 padded rows
NBLK = C_IN + C_OUT         # 68 56-wide column blocks (64 padded-x + 4 y)
COLS = NBLK * WV            # 3808 columns of the packed input
NUK = 7                     # computed u-shifts 0..6 (7,8 come from symmetry)
UF = 5                      # u-shifts computed for the x-x correlation
GW = 32 * NU                # 288 columns per x-corr accumulation group
GY = C_IN * DS              # 320 columns per y-corr accumulation group
NCORES = 8

_CACHED = {}
_TIMING = True


def _mark(t, name):
    if _TIMING:
        now = time.perf_counter()
        print(f"[phase] {name}: {now - t[0]:.3f}s", file=sys.stderr)
        t[0] = now


def _build_nc():
    import concourse.bass as bass
    import concourse.mybir as mybir
    from concourse.tile import TileContext

    nc = bass.Bass()
    inp = nc.dram_tensor("inp", [HP, COLS], mybir.dt.bfloat16, kind="ExternalInput")
    out1 = nc.dram_tensor("o1", [C_IN, UF * 2 * GW], mybir.dt.bfloat16, kind="ExternalOutput")
    out2 = nc.dram_tensor("o2", [C_OUT, DS * GY], mybir.dt.bfloat16, kind="ExternalOutput")

    with TileContext(nc) as tc:
        with (
            tc.tile_pool(name="inp_p", bufs=1) as inp_p,
            tc.tile_pool(name="ps_p", bufs=6, space="PSUM") as ps_p,
            tc.tile_pool(name="py_p", bufs=2, space="PSUM") as py_p,
            tc.tile_pool(name="st_p", bufs=1) as st_p,
        ):
            # One DMA materializes all 7 u-shifted replicas via an
            # overlapping sliding-window source AP: all_t[h, u, b, w] =
            # inp[h+u, b, w].  A single DMA completion sem keeps every
            # matmul at <=1 attached sync wait (the HW limit).  The
            # unpadded x itself (the matmul lhsT) is the interior of the
            # u=4 replica, so x is shipped only once.
            all_t = inp_p.tile([H, NUK, NBLK, WV], mybir.dt.bfloat16)
            src = inp[:, :]
            v = src.ap
            v.clear()
            v.extend([(COLS, H), (COLS, NUK), (WV, NBLK), (1, WV)])
            nc.sync.dma_start(out=all_t[:, :, :, :], in_=src)

            stage1 = st_p.tile([C_IN, UF * 2 * GW], mybir.dt.bfloat16)
            stage2 = st_p.tile([C_OUT, DS * GY], mybir.dt.bfloat16)
            # x-x correlation: corr[j, i, u, v], u in 0..4 (rest by symmetry)
            for u in range(UF):
                for ihalf in range(2):
                    ps = ps_p.tile([C_IN, GW], mybir.dt.float32)
                    for wl in range(WH):
                        nc.tensor.matmul(
                            ps[:, :],
                            all_t[:, 4, 0:C_IN, wl + 4],
                            all_t[:, u, ihalf * 32:(ihalf + 1) * 32, wl:wl + NU],
                            start=(wl == 0),
                            stop=(wl == WH - 1),
                        )
                    col = (u * 2 + ihalf) * GW
                    nc.vector.tensor_copy(stage1[:, col:col + GW], ps[:, :])
            # y-x correlation: p2[co, i, u, v], u in 2..6, v in 2..6
            for ui in range(DS):
                psy = py_p.tile([C_OUT, GY], mybir.dt.float32)
                for wl in range(WH):
                    nc.tensor.matmul(
                        psy[:, :],
                        all_t[:, 0, C_IN:NBLK, wl],
                        all_t[:, ui + 2, 0:C_IN, wl + 2:wl + 2 + DS],
                        start=(wl == 0),
                        stop=(wl == WH - 1),
                    )
                nc.vector.tensor_copy(stage2[:, ui * GY:(ui + 1) * GY], psy[:, :])
            nc.sync.dma_start(out=out1[:, :], in_=stage1[:, :])
            nc.sync.dma_start(out=out2[:, :], in_=stage2[:, :])

    _split_multiwait_drains(nc)
    return nc


def _split_multiwait_drains(nc):
    """Walrus rejects instructions carrying more than one attached sync wait.

    Tile's kernel-tail drain waits on every outstanding semaphore in one
    instruction; split it into a chain of single-wait drains.
    """
    import copy

    import concourse.mybir as mybir

    for fobj in nc.m.functions:
        for blk in fobj.blocks:
            insts = blk.instructions
            k = 0
            while k < len(insts):
                inst = insts[k]
                si = inst.sync_info
                if (
                    isinstance(inst, mybir.InstDrain)
                    and si is not None
                    and len(si.on_wait) > 1
                ):
                    waits = list(si.on_wait)
                    for j, w in enumerate(waits[:-1]):
                        d = copy.copy(inst)
                        d.name = f"{inst.name}_w{j}"
                        d.sync_info = mybir.SyncInfo(on_wait=[w], on_update=[])
                        nc.register_instruction(d)
                        insts.insert(k, d)
                        k += 1
                    inst.sync_info = mybir.SyncInfo(
                        on_wait=[waits[-1]], on_update=list(si.on_update)
                    )
                k += 1


def _build_runner():
    """Build the bass module once and return a cached jitted SPMD callable.

    Mirrors bass2jax.run_bass_via_pjrt's multi-core path, but the jitted
    shard_map is constructed a single time so later calls skip
    trace/lower/compile entirely.
    """
    import jax
    import concourse.mybir as mybir
    from concourse.bass2jax import (
        _bass_exec_p,
        install_neuronx_cc_hook,
        partition_id_tensor,
    )
    from jax.experimental.shard_map import shard_map
    from jax.sharding import Mesh, PartitionSpec

    nc = _build_nc()
    if not nc.is_finalized():
        nc.finalize()
    install_neuronx_cc_hook()
    assert nc.dbg_addr is None
    partition_name = (
        nc.partition_id_tensor.name if nc.partition_id_tensor is not None else None
    )

    in_names, out_names, out_avals, zero_shapes = [], [], [], []
    for alloc in nc.m.functions[0].allocations:
        if not isinstance(alloc, mybir.MemoryLocationSet):
            continue
        name = alloc.memorylocations[0].name
        if alloc.kind == "ExternalInput":
            if name != partition_name:
                in_names.append(name)
        elif alloc.kind == "ExternalOutput":
            shape = tuple(alloc.tensor_shape)
            dtype = mybir.dt.np(alloc.dtype)
            out_names.append(name)
            out_avals.append(jax.core.ShapedArray(shape, dtype))
            zero_shapes.append((shape, dtype))
    n_params = len(in_names)
    n_outs = len(out_avals)
    all_names = in_names + out_names
    if partition_name is not None:
        all_names = all_names + [partition_name]

    def _body(*args):
        operands = list(args)
        if partition_name is not None:
            operands.append(partition_id_tensor())
        outs = _bass_exec_p.bind(
            *operands,
            out_avals=tuple(out_avals),
            in_names=tuple(all_names),
            out_names=tuple(out_names),
            lowering_input_output_aliases=(),
            sim_require_finite=True,
            sim_require_nnan=True,
            nc=nc,
        )
        return tuple(outs)

    devices = jax.devices()[:NCORES]
    mesh = Mesh(np.asarray(devices), ("core",))
    donate = tuple(range(n_params, n_params + n_outs))
    sharded = jax.jit(
        shard_map(
            _body,
            mesh=mesh,
            in_specs=(PartitionSpec("core"),) * (n_params + n_outs),
            out_specs=(PartitionSpec("core"),) * n_outs,
            check_rep=False,
        ),
        donate_argnums=donate,
        keep_unused=True,
    )

    # The donated output-seed buffers never leave the device: a jitted
    # sharded zeros-maker replaces an 11MB host->device upload per call.
    import jax.numpy as jnp
    from jax.sharding import NamedSharding

    zeros_sharding = tuple(
        NamedSharding(mesh, PartitionSpec("core")) for _ in zero_shapes
    )
    zeros_fn = jax.jit(
        lambda: tuple(
            jnp.zeros((NCORES * s[0], *s[1:]), dt) for s, dt in zero_shapes
        ),
        out_shardings=zeros_sharding,
    )

    def run(in_maps):
        t = [time.perf_counter()]
        concat_in = [
            np.concatenate([np.asarray(m[name]) for m in in_maps], axis=0)
            for name in in_names
        ]
        zeros = zeros_fn()
        _mark(t, "  run.concat")
        in_shardings = [
            NamedSharding(mesh, PartitionSpec("core")) for _ in concat_in
        ]
        dev_in = jax.device_put(concat_in, in_shardings)
        for a in dev_in:
            a.block_until_ready()
        _mark(t, "  run.upload")
        out_arrs = sharded(*dev_in, *zeros)
        _mark(t, "  run.dispatch")
        for a in out_arrs:
            a.block_until_ready()
        _mark(t, "  run.exec")
        jobs = []
        for i, a in enumerate(out_arrs):
            rows = out_avals[i].shape[0]
            for sh in a.addressable_shards:
                c = sh.index[0].start // rows if sh.index[0].start else 0
                sh.data.copy_to_host_async()
                jobs.append((i, c, sh.data))

        res = [
            np.empty((NCORES, *out_avals[i].shape), out_avals[i].dtype)
            for i in range(n_outs)
        ]
        for i, c, data in jobs:
            res[i][c] = np.asarray(data).reshape(out_avals[i].shape)
        _mark(t, "  run.fetch")
        return res

    return run


def _unfold(x1):
    """x1: [C_in, H, W] -> U [10000, 1600] (kept for test.py's oracle)."""
    from numpy.lib.stride_tricks import sliding_window_view

    xp2 = np.pad(x1, ((0, 0), (4, 4), (4, 4)))
    sw = sliding_window_view(xp2, (DS, DS), axis=(1, 2))
    return np.ascontiguousarray(
        sw.transpose(1, 2, 0, 3, 4).reshape(100 * 100, K), dtype=np.float32
    )


def _prep_in_maps(x, y):
    import ml_dtypes

    bf16 = ml_dtypes.bfloat16
    in_maps = []
    for s in range(N):
        xs = x[s, 0]
        ys = y[s, :, 0]
        yT = ys.transpose(1, 0, 2)                              # [96, 4, 96]
        xpad = np.zeros((C_IN, HP, HP), np.float32)
        xpad[:, DS - 1:DS - 1 + H, DS - 1:DS - 1 + W] = xs
        xpfT = xpad.transpose(1, 0, 2)                          # [104, 64, 104]
        for half in range(2):
            packed = np.zeros((HP, NBLK, WV), np.float32)
            packed[:, :C_IN, :] = xpfT[:, :, WH * half:WH * half + WV]
            packed[:H, C_IN:, :WH] = yT[:, :, WH * half:WH * (half + 1)]
            in_maps.append({"inp": packed.reshape(HP, COLS).astype(bf16)})
    return in_maps


def kernel(x, d, y, alpha, reg):
    from numpy.lib.stride_tricks import sliding_window_view
    from scipy.linalg import cho_factor, cho_solve

    t = [time.perf_counter()]
    x = np.asarray(x, dtype=np.float32)
    d = np.asarray(d, dtype=np.float32)
    y = np.asarray(y, dtype=np.float32)
    alpha = np.asarray(alpha, dtype=np.float32)
    reg = np.asarray(reg, dtype=np.float32)

    if "run" not in _CACHED:
        _CACHED["run"] = _build_runner()
    run = _CACHED["run"]
    _mark(t, "build")

    in_maps = _prep_in_maps(x, y)
    _mark(t, "prep")

    res1, res2 = run(in_maps)            # [8, 68, 2880] bf16, [8, 4, 1152] bf16
    _mark(t, "spmd_run")

    a = alpha.reshape(N) * H * W * float(reg[0]) / (DS * DS * C_IN)
    out = np.empty((N, C_OUT, C_IN, DS, DS), dtype=np.float32)

    tp = {}

    def _tp(key, t0):
        tp[key] = tp.get(key, 0.0) + time.perf_counter() - t0
        return time.perf_counter()

    def _solve(s):
        t0 = time.perf_counter()
        o1 = np.asarray(res1[2 * s], np.float32) + np.asarray(res1[2 * s + 1], np.float32)
        o2 = np.asarray(res2[2 * s], np.float32) + np.asarray(res2[2 * s + 1], np.float32)
        t0 = _tp("cvt", t0)
        # o1 columns are (u<5, ihalf, i_local, v) -> [j, i, u, v]
        cl = np.ascontiguousarray(
            o1.reshape(C_IN, UF, 2, 32, NU).transpose(0, 2, 3, 1, 4)
        ).reshape(C_IN, C_IN, UF, NU)
        # corr[j,i,u,v]; u>=5 from symmetry corr[j,i,u,v] = corr[i,j,8-u,8-v]
        corr = np.empty((C_IN, C_IN, NU, NU), np.float32)
        corr[:, :, :UF, :] = cl
        corr[:, :, UF:, :] = np.flip(
            cl.transpose(1, 0, 2, 3)[:, :, :NU - UF, :], axis=(2, 3)
        )

        # Q[(j,kh,kw),(i,ph,pw)] = corr[j, i, ph-kh+4, pw-kw+4]
        swv = sliding_window_view(corr, (DS, DS), axis=(2, 3))   # [j,i,a,b,ph,pw]
        Q4 = swv[:, :, ::-1, ::-1, :, :].transpose(0, 2, 3, 1, 4, 5)
        Q = np.ascontiguousarray(Q4).reshape(K, K)
        Q.flat[::K + 1] += a[s]
        t0 = _tp("qgather", t0)

        # o2 columns are (u-2, i, v-2) for u,v in 2..6 -> P[(i,ph,pw), co]
        p2u = o2.reshape(C_OUT, DS, C_IN, DS)
        P = np.ascontiguousarray(p2u.transpose(2, 1, 3, 0)).reshape(K, C_OUT)
        P += a[s] * d[s].transpose(1, 2, 3, 0).reshape(K, C_OUT)
        t0 = _tp("pprep", t0)

        cf = cho_factor(Q, lower=False, check_finite=False)
        D = cho_solve(cf, P, check_finite=False)
        t0 = _tp("chol", t0)
        out[s] = D.reshape(C_IN, DS, DS, C_OUT).transpose(3, 0, 1, 2)

    from concurrent.futures import ThreadPoolExecutor

    with ThreadPoolExecutor(max_workers=N) as ex:
        list(ex.map(_solve, range(N)))
    if _TIMING:
        print(f"[phase]   post breakdown: {tp}", file=sys.stderr)
    _mark(t, "host_post")
    return out


# revision 40
# speedup vs baseline: 1.0967x; 1.0967x over previous
"""Bass/Trainium2 kernel for nn_DCDicl (DSBlock forward).

Algorithm: instead of the O(K^2 * R) unfold-Gram (baseline), compute the
all-pairs shift correlation corr[j,i,u,v] = sum_{h,w} x[j,h,w] *
xpad[i,h+u-4,w+v-4] (8x fewer FLOPs — the Gram is a Toeplitz gather of
corr), plus the U^T y rows for P folded into the same matmuls.

Device (8 cores = 4 samples x 2 w-halves, bf16 in / fp32 psum):
  out[m, (u,i,v)] = sum_{h, w in half} XY[m,h,w] * xpad[i, h+u, w+v]
with contraction over h (96 partitions) and PSUM accumulation over w.
Host: sum halves, gather Q via a sliding-window view, fp32 Cholesky solve.
"""

import sys
import time

import numpy as np

if "/opt/trn_rl_repo" not in sys.path:
    sys.path.append("/opt/trn_rl_repo")

N, C_IN, C_OUT, H, W, DS = 4, 64, 4, 96, 96, 5
K = C_IN * DS * DS          # 1600
NU = 2 * DS - 1             # 9 shifts per axis
M = C_IN + C_OUT            # 68 lhs rows (64 x-channels + 4 y-channels)
WH = W // 2                 # 48 w-columns per core (contraction half)
WV = WH + NU - 1            # 56 w-columns of padded image needed per core
HP = H + 2 * (DS - 1)       # 104 padded rows
NBLK = C_IN + C_OUT         # 68 56-wide column blocks (64 padded-x + 4 y)
COLS = NBLK * WV            # 3808 columns of the packed input
NUK = 7                     # computed u-shifts 0..6 (7,8 come from symmetry)
UF = 5                      # u-shifts computed for the x-x correlation
GW = 32 * NU                # 288 columns per x-corr accumulation group
GY = C_IN * DS              # 320 columns per y-corr accumulation group
NCORES = 8

_CACHED = {}
_TIMING = True


def _mark(t, name):
    if _TIMING:
        now = time.perf_counter()
        print(f"[phase] {name}: {now - t[0]:.3f}s", file=sys.stderr)
        t[0] = now


def _build_nc():
    import concourse.bass as bass
    import concourse.mybir as mybir
    from concourse.tile import TileContext

    nc = bass.Bass()
    inp = nc.dram_tensor("inp", [HP, COLS], mybir.dt.bfloat16, kind="ExternalInput")
    out1 = nc.dram_tensor(
        "o1", [C_IN, UF * 2 * GW + DS * GY], mybir.dt.bfloat16, kind="ExternalOutput"
    )

    with TileContext(nc) as tc:
        with (
            tc.tile_pool(name="inp_p", bufs=1) as inp_p,
            tc.tile_pool(name="ps_p", bufs=6, space="PSUM") as ps_p,
            tc.tile_pool(name="py_p", bufs=2, space="PSUM") as py_p,
            tc.tile_pool(name="st_p", bufs=1) as st_p,
        ):
            # One DMA materializes all 7 u-shifted replicas via an
            # overlapping sliding-window source AP: all_t[h, u, b, w] =
            # inp[h+u, b, w].  A single DMA completion sem keeps every
            # matmul at <=1 attached sync wait (the HW limit).  The
            # unpadded x itself (the matmul lhsT) is the interior of the
            # u=4 replica, so x is shipped only once.
            all_t = inp_p.tile([H, NUK, NBLK, WV], mybir.dt.bfloat16)
            src = inp[:, :]
            v = src.ap
            v.clear()
            v.extend([(COLS, H), (COLS, NUK), (WV, NBLK), (1, WV)])
            nc.sync.dma_start(out=all_t[:, :, :, :], in_=src)

            stage1 = st_p.tile([C_IN, UF * 2 * GW + DS * GY], mybir.dt.bfloat16)
            nc.vector.memset(stage1[:, :], 0)
            # x-x correlation: corr[j, i, u, v], u in 0..4 (rest by symmetry)
            for u in range(UF):
                for ihalf in range(2):
                    ps = ps_p.tile([C_IN, GW], mybir.dt.float32)
                    for wl in range(WH):
                        nc.tensor.matmul(
                            ps[:, :],
                            all_t[:, 4, 0:C_IN, wl + 4],
                            all_t[:, u, ihalf * 32:(ihalf + 1) * 32, wl:wl + NU],
                            start=(wl == 0),
                            stop=(wl == WH - 1),
                        )
                    col = (u * 2 + ihalf) * GW
                    nc.vector.tensor_copy(stage1[:, col:col + GW], ps[:, :])
            # y-x correlation: p2[co, i, u, v], u in 2..6, v in 2..6
            for ui in range(DS):
                psy = py_p.tile([C_OUT, GY], mybir.dt.float32)
                for wl in range(WH):
                    nc.tensor.matmul(
                        psy[:, :],
                        all_t[:, 0, C_IN:NBLK, wl],
                        all_t[:, ui + 2, 0:C_IN, wl + 2:wl + 2 + DS],
                        start=(wl == 0),
                        stop=(wl == WH - 1),
                    )
                col = UF * 2 * GW + ui * GY
                nc.vector.tensor_copy(stage1[:C_OUT, col:col + GY], psy[:, :])
            nc.sync.dma_start(out=out1[:, :], in_=stage1[:, :])

    _split_multiwait(nc)
    return nc


def _split_multiwait(nc):
    """Walrus rejects instructions carrying more than one attached sync wait.

    For any instruction with N>1 waits, hoist N-1 of them onto same-engine
    NoOps inserted immediately before it.
    """
    import concourse.mybir as mybir

    for fobj in nc.m.functions:
        for blk in fobj.blocks:
            insts = blk.instructions
            k = 0
            while k < len(insts):
                inst = insts[k]
                si = inst.sync_info
                if si is not None and len(si.on_wait) > 1:
                    waits = list(si.on_wait)
                    for j, w in enumerate(waits[:-1]):
                        d = mybir.InstNoOp(
                            name=f"{inst.name}_w{j}",
                            engine=inst.engine,
                            bass_nofuse=True,
                            sync_info=mybir.SyncInfo(on_wait=[w], on_update=[]),
                        )
                        nc.register_instruction(d)
                        insts.insert(k, d)
                        k += 1
                    inst.sync_info = mybir.SyncInfo(
                        on_wait=[waits[-1]], on_update=list(si.on_update)
                    )
                k += 1


def _build_runner():
    """Build the bass module once and return a cached jitted SPMD callable.

    Mirrors bass2jax.run_bass_via_pjrt's multi-core path, but the jitted
    shard_map is constructed a single time so later calls skip
    trace/lower/compile entirely.
    """
    import jax
    import concourse.mybir as mybir
    from concourse.bass2jax import (
        _bass_exec_p,
        install_neuronx_cc_hook,
        partition_id_tensor,
    )
    from jax.experimental.shard_map import shard_map
    from jax.sharding import Mesh, PartitionSpec

    nc = _build_nc()
    if not nc.is_finalized():
        nc.finalize()
    install_neuronx_cc_hook()
    assert nc.dbg_addr is None
    partition_name = (
        nc.partition_id_tensor.name if nc.partition_id_tensor is not None else None
    )

    in_names, out_names, out_avals, zero_shapes = [], [], [], []
    for alloc in nc.m.functions[0].allocations:
        if not isinstance(alloc, mybir.MemoryLocationSet):
            continue
        name = alloc.memorylocations[0].name
        if alloc.kind == "ExternalInput":
            if name != partition_name:
                in_names.append(name)
        elif alloc.kind == "ExternalOutput":
            shape = tuple(alloc.tensor_shape)
            dtype = mybir.dt.np(alloc.dtype)
            out_names.append(name)
            out_avals.append(jax.core.ShapedArray(shape, dtype))
            zero_shapes.append((shape, dtype))
    n_params = len(in_names)
    n_outs = len(out_avals)
    all_names = in_names + out_names
    if partition_name is not None:
        all_names = all_names + [partition_name]

    def _body(*args):
        operands = list(args)
        if partition_name is not None:
            operands.append(partition_id_tensor())
        outs = _bass_exec_p.bind(
            *operands,
            out_avals=tuple(out_avals),
            in_names=tuple(all_names),
            out_names=tuple(out_names),
            lowering_input_output_aliases=(),
            sim_require_finite=True,
            sim_require_nnan=True,
            nc=nc,
        )
        return tuple(outs)

    devices = jax.devices()[:NCORES]
    mesh = Mesh(np.asarray(devices), ("core",))
    donate = tuple(range(n_params, n_params + n_outs))
    sharded = jax.jit(
        shard_map(
            _body,
            mesh=mesh,
            in_specs=(PartitionSpec("core"),) * (n_params + n_outs),
            out_specs=(PartitionSpec("core"),) * n_outs,
            check_rep=False,
        ),
        donate_argnums=donate,
        keep_unused=True,
    )

    # The donated output-seed buffers never leave the device: a jitted
    # sharded zeros-maker replaces an 11MB host->device upload per call.
    import jax.numpy as jnp
    from jax.sharding import NamedSharding

    zeros_sharding = tuple(
        NamedSharding(mesh, PartitionSpec("core")) for _ in zero_shapes
    )
    zeros_fn = jax.jit(
        lambda: tuple(
            jnp.zeros((NCORES * s[0], *s[1:]), dt) for s, dt in zero_shapes
        ),
        out_shardings=zeros_sharding,
    )

    from concurrent.futures import ThreadPoolExecutor

    in_sharding = NamedSharding(mesh, PartitionSpec("core"))

    def run(in_maps):
        t = [time.perf_counter()]
        per_core = [
            [np.ascontiguousarray(m[name]) for m in in_maps] for name in in_names
        ]
        zeros = zeros_fn()
        _mark(t, "  run.concat")

        def _up(job):
            i, c = job
            return i, c, jax.device_put(per_core[i][c], devices[c])

        singles = [[None] * NCORES for _ in range(n_params)]
        jobs = [(i, c) for i in range(n_params) for c in range(NCORES)]
        with ThreadPoolExecutor(max_workers=NCORES) as ex:
            for i, c, arr in ex.map(_up, jobs):
                singles[i][c] = arr
        dev_in = [
            jax.make_array_from_single_device_arrays(
                (NCORES * per_core[i][0].shape[0], *per_core[i][0].shape[1:]),
                in_sharding,
                singles[i],
            )
            for i in range(n_params)
        ]
        _mark(t, "  run.upload")
        out_arrs = sharded(*dev_in, *zeros)
        _mark(t, "  run.dispatch")
        for a in out_arrs:
            a.block_until_ready()
        _mark(t, "  run.exec")
        fjobs = []
        for i, a in enumerate(out_arrs):
            rows = out_avals[i].shape[0]
            for sh in a.addressable_shards:
                c = sh.index[0].start // rows if sh.index[0].start else 0
                fjobs.append((i, c, sh.data))

        def _fetch(job):
            i, c, data = job
            return i, c, np.asarray(data)

        res = [
            np.empty((NCORES, *out_avals[i].shape), out_avals[i].dtype)
            for i in range(n_outs)
        ]
        with ThreadPoolExecutor(max_workers=NCORES) as ex:
            for i, c, arr in ex.map(_fetch, fjobs):
                res[i][c] = arr.reshape(out_avals[i].shape)
        _mark(t, "  run.fetch")
        return res

    return run


def _unfold(x1):
    """x1: [C_in, H, W] -> U [10000, 1600] (kept for test.py's oracle)."""
    from numpy.lib.stride_tricks import sliding_window_view

    xp2 = np.pad(x1, ((0, 0), (4, 4), (4, 4)))
    sw = sliding_window_view(xp2, (DS, DS), axis=(1, 2))
    return np.ascontiguousarray(
        sw.transpose(1, 2, 0, 3, 4).reshape(100 * 100, K), dtype=np.float32
    )


def _prep_in_maps(x, y):
    import ml_dtypes

    bf16 = ml_dtypes.bfloat16
    in_maps = []
    for s in range(N):
        xs = x[s, 0]
        ys = y[s, :, 0]
        yT = ys.transpose(1, 0, 2)                              # [96, 4, 96]
        xpad = np.zeros((C_IN, HP, HP), np.float32)
        xpad[:, DS - 1:DS - 1 + H, DS - 1:DS - 1 + W] = xs
        xpfT = xpad.transpose(1, 0, 2)                          # [104, 64, 104]
        for half in range(2):
            packed = np.zeros((HP, NBLK, WV), np.float32)
            packed[:, :C_IN, :] = xpfT[:, :, WH * half:WH * half + WV]
            packed[:H, C_IN:, :WH] = yT[:, :, WH * half:WH * (half + 1)]
            in_maps.append({"inp": packed.reshape(HP, COLS).astype(bf16)})
    return in_maps


def kernel(x, d, y, alpha, reg):
    from numpy.lib.stride_tricks import sliding_window_view

    t = [time.perf_counter()]
    x = np.asarray(x, dtype=np.float32)
    d = np.asarray(d, dtype=np.float32)
    y = np.asarray(y, dtype=np.float32)
    alpha = np.asarray(alpha, dtype=np.float32)
    reg = np.asarray(reg, dtype=np.float32)

    if "run" not in _CACHED:
        _CACHED["run"] = _build_runner()
    run = _CACHED["run"]
    _mark(t, "build")

    in_maps = _prep_in_maps(x, y)
    _mark(t, "prep")

    res1 = run(in_maps)[0]               # [8, 64, 2880 + 1600] bf16
    _mark(t, "spmd_run")

    a = alpha.reshape(N) * H * W * float(reg[0]) / (DS * DS * C_IN)
    Qs = np.empty((N, K, K), np.float32)
    Ps = np.empty((N, K, C_OUT), np.float32)

    def _gather(s):
        o = np.asarray(res1[2 * s], np.float32) + np.asarray(res1[2 * s + 1], np.float32)
        o1 = o[:, :UF * 2 * GW]
        o2 = o[:C_OUT, UF * 2 * GW:]
        # o1 columns are (u<5, ihalf, i_local, v) -> [j, i, u, v]
        cl = np.ascontiguousarray(
            o1.reshape(C_IN, UF, 2, 32, NU).transpose(0, 2, 3, 1, 4)
        ).reshape(C_IN, C_IN, UF, NU)
        # corr[j,i,u,v]; u>=5 from symmetry corr[j,i,u,v] = corr[i,j,8-u,8-v]
        corr = np.empty((C_IN, C_IN, NU, NU), np.float32)
        corr[:, :, :UF, :] = cl
        corr[:, :, UF:, :] = np.flip(
            cl.transpose(1, 0, 2, 3)[:, :, :NU - UF, :], axis=(2, 3)
        )

        # Q[(j,kh,kw),(i,ph,pw)] = corr[j, i, ph-kh+4, pw-kw+4]
        swv = sliding_window_view(corr, (DS, DS), axis=(2, 3))   # [j,i,a,b,ph,pw]
        Q4 = swv[:, :, ::-1, ::-1, :, :].transpose(0, 2, 3, 1, 4, 5)
        Q = Qs[s].reshape(C_IN, DS, DS, C_IN, DS, DS)
        np.copyto(Q, Q4)
        Qs[s].flat[::K + 1] += a[s]

        # o2 columns are (u-2, i, v-2) for u,v in 2..6 -> P[(i,ph,pw), co]
        p2u = o2.reshape(C_OUT, DS, C_IN, DS)
        np.copyto(Ps[s].reshape(C_IN, DS, DS, C_OUT), p2u.transpose(2, 1, 3, 0))
        Ps[s] += a[s] * d[s].transpose(1, 2, 3, 0).reshape(K, C_OUT)

    from concurrent.futures import ThreadPoolExecutor

    with ThreadPoolExecutor(max_workers=N) as ex:
        list(ex.map(_gather, range(N)))
    _mark(t, "host_gather")

    # Batched conjugate gradient: Q is SPD with kappa ~ 6, so 24 iterations
    # reach ~1e-7 relative residual -- far below the bf16 noise floor.
    X = np.zeros((N, K, C_OUT), np.float32)
    R = Ps.copy()
    Pc = R.copy()
    rs = np.einsum("nkc,nkc->nc", R, R)
    for _ in range(24):
        QP = np.matmul(Qs, Pc)
        al = (rs / np.einsum("nkc,nkc->nc", Pc, QP))[:, None, :]
        X += Pc * al
        R -= QP * al
        rs_new = np.einsum("nkc,nkc->nc", R, R)
        Pc = R + (rs_new / rs)[:, None, :] * Pc
        rs = rs_new
    out = np.ascontiguousarray(
        X.reshape(N, C_IN, DS, DS, C_OUT).transpose(0, 4, 1, 2, 3)
    )
    _mark(t, "host_solve")
    return out


# revision 44
# speedup vs baseline: 1.6617x; 1.5151x over previous
"""Bass/Trainium2 kernel for nn_DCDicl (DSBlock forward).

Algorithm: instead of the O(K^2 * R) unfold-Gram (baseline), compute the
all-pairs shift correlation corr[j,i,u,v] = sum_{h,w} x[j,h,w] *
xpad[i,h+u-4,w+v-4] (8x fewer FLOPs — the Gram is a Toeplitz gather of
corr), plus the U^T y rows for P folded into the same matmuls.

Device (8 cores = 4 samples x 2 w-halves, bf16 in / fp32 psum):
  out[m, (u,i,v)] = sum_{h, w in half} XY[m,h,w] * xpad[i, h+u, w+v]
with contraction over h (96 partitions) and PSUM accumulation over w.
Host: sum halves, gather Q via a sliding-window view, fp32 Cholesky solve.
"""

import sys
import time

import numpy as np

if "/opt/trn_rl_repo" not in sys.path:
    sys.path.append("/opt/trn_rl_repo")

N, C_IN, C_OUT, H, W, DS = 4, 64, 4, 96, 96, 5
K = C_IN * DS * DS          # 1600
NU = 2 * DS - 1             # 9 shifts per axis
M = C_IN + C_OUT            # 68 lhs rows (64 x-channels + 4 y-channels)
WH = W // 2                 # 48 w-columns per core (contraction half)
WV = WH + NU - 1            # 56 w-columns of padded image needed per core
HP = H + 2 * (DS - 1)       # 104 padded rows
NBLK = C_IN + C_OUT         # 68 56-wide column blocks (64 padded-x + 4 y)
COLS = NBLK * WV            # 3808 columns of the packed input
NUK = 7                     # computed u-shifts 0..6 (7,8 come from symmetry)
UF = 5                      # u-shifts computed for the x-x correlation
GW = 32 * NU                # 288 columns per x-corr accumulation group
GY = C_IN * DS              # 320 columns per y-corr accumulation group
NCORES = 8

_CACHED = {}
_TIMING = True


def _mark(t, name):
    if _TIMING:
        now = time.perf_counter()
        print(f"[phase] {name}: {now - t[0]:.3f}s", file=sys.stderr)
        t[0] = now


def _build_nc():
    import concourse.bass as bass
    import concourse.mybir as mybir
    from concourse.tile import TileContext

    nc = bass.Bass()
    inp = nc.dram_tensor("inp", [HP, COLS], mybir.dt.bfloat16, kind="ExternalInput")
    out1 = nc.dram_tensor(
        "o1", [C_IN, UF * 2 * GW + DS * GY], mybir.dt.bfloat16, kind="ExternalOutput"
    )

    with TileContext(nc) as tc:
        with (
            tc.tile_pool(name="inp_p", bufs=1) as inp_p,
            tc.tile_pool(name="ps_p", bufs=6, space="PSUM") as ps_p,
            tc.tile_pool(name="py_p", bufs=2, space="PSUM") as py_p,
            tc.tile_pool(name="st_p", bufs=1) as st_p,
        ):
            # One DMA materializes all 7 u-shifted replicas via an
            # overlapping sliding-window source AP: all_t[h, u, b, w] =
            # inp[h+u, b, w].  A single DMA completion sem keeps every
            # matmul at <=1 attached sync wait (the HW limit).  The
            # unpadded x itself (the matmul lhsT) is the interior of the
            # u=4 replica, so x is shipped only once.
            all_t = inp_p.tile([H, NUK, NBLK, WV], mybir.dt.bfloat16)
            src = inp[:, :]
            v = src.ap
            v.clear()
            v.extend([(COLS, H), (COLS, NUK), (WV, NBLK), (1, WV)])
            nc.sync.dma_start(out=all_t[:, :, :, :], in_=src)

            stage1 = st_p.tile([C_IN, UF * 2 * GW + DS * GY], mybir.dt.bfloat16)
            nc.vector.memset(stage1[:, :], 0)
            # x-x correlation: corr[j, i, u, v], u in 0..4 (rest by symmetry)
            for u in range(UF):
                for ihalf in range(2):
                    ps = ps_p.tile([C_IN, GW], mybir.dt.float32)
                    for wl in range(WH):
                        nc.tensor.matmul(
                            ps[:, :],
                            all_t[:, 4, 0:C_IN, wl + 4],
                            all_t[:, u, ihalf * 32:(ihalf + 1) * 32, wl:wl + NU],
                            start=(wl == 0),
                            stop=(wl == WH - 1),
                        )
                    col = (u * 2 + ihalf) * GW
                    nc.vector.tensor_copy(stage1[:, col:col + GW], ps[:, :])
            # y-x correlation: p2[co, i, u, v], u in 2..6, v in 2..6
            for ui in range(DS):
                psy = py_p.tile([C_OUT, GY], mybir.dt.float32)
                for wl in range(WH):
                    nc.tensor.matmul(
                        psy[:, :],
                        all_t[:, 0, C_IN:NBLK, wl],
                        all_t[:, ui + 2, 0:C_IN, wl + 2:wl + 2 + DS],
                        start=(wl == 0),
                        stop=(wl == WH - 1),
                    )
                col = UF * 2 * GW + ui * GY
                nc.vector.tensor_copy(stage1[:C_OUT, col:col + GY], psy[:, :])
            nc.sync.dma_start(out=out1[:, :], in_=stage1[:, :])

    _split_multiwait(nc)
    return nc


def _split_multiwait(nc):
    """Walrus rejects instructions carrying more than one attached sync wait.

    For any instruction with N>1 waits, hoist N-1 of them onto same-engine
    NoOps inserted immediately before it.
    """
    import concourse.mybir as mybir

    for fobj in nc.m.functions:
        for blk in fobj.blocks:
            insts = blk.instructions
            k = 0
            while k < len(insts):
                inst = insts[k]
                si = inst.sync_info
                if si is not None and len(si.on_wait) > 1:
                    waits = list(si.on_wait)
                    for j, w in enumerate(waits[:-1]):
                        d = mybir.InstNoOp(
                            name=f"{inst.name}_w{j}",
                            engine=inst.engine,
                            bass_nofuse=True,
                            sync_info=mybir.SyncInfo(on_wait=[w], on_update=[]),
                        )
                        nc.register_instruction(d)
                        insts.insert(k, d)
                        k += 1
                    inst.sync_info = mybir.SyncInfo(
                        on_wait=[waits[-1]], on_update=list(si.on_update)
                    )
                k += 1


def _build_runner():
    """Build the bass module once and return a cached jitted SPMD callable.

    Mirrors bass2jax.run_bass_via_pjrt's multi-core path, but the jitted
    shard_map is constructed a single time so later calls skip
    trace/lower/compile entirely.
    """
    import jax
    import concourse.mybir as mybir
    from concourse.bass2jax import (
        _bass_exec_p,
        install_neuronx_cc_hook,
        partition_id_tensor,
    )
    from jax.experimental.shard_map import shard_map
    from jax.sharding import Mesh, PartitionSpec

    nc = _build_nc()
    if not nc.is_finalized():
        nc.finalize()
    install_neuronx_cc_hook()
    assert nc.dbg_addr is None
    partition_name = (
        nc.partition_id_tensor.name if nc.partition_id_tensor is not None else None
    )

    in_names, out_names, out_avals, zero_shapes = [], [], [], []
    for alloc in nc.m.functions[0].allocations:
        if not isinstance(alloc, mybir.MemoryLocationSet):
            continue
        name = alloc.memorylocations[0].name
        if alloc.kind == "ExternalInput":
            if name != partition_name:
                in_names.append(name)
        elif alloc.kind == "ExternalOutput":
            shape = tuple(alloc.tensor_shape)
            dtype = mybir.dt.np(alloc.dtype)
            out_names.append(name)
            out_avals.append(jax.core.ShapedArray(shape, dtype))
            zero_shapes.append((shape, dtype))
    n_params = len(in_names)
    n_outs = len(out_avals)
    all_names = in_names + out_names
    if partition_name is not None:
        all_names = all_names + [partition_name]

    def _body(*args):
        operands = list(args)
        if partition_name is not None:
            operands.append(partition_id_tensor())
        outs = _bass_exec_p.bind(
            *operands,
            out_avals=tuple(out_avals),
            in_names=tuple(all_names),
            out_names=tuple(out_names),
            lowering_input_output_aliases=(),
            sim_require_finite=True,
            sim_require_nnan=True,
            nc=nc,
        )
        return tuple(outs)

    devices = jax.devices()[:NCORES]
    mesh = Mesh(np.asarray(devices), ("core",))
    donate = tuple(range(n_params, n_params + n_outs))
    sharded = jax.jit(
        shard_map(
            _body,
            mesh=mesh,
            in_specs=(PartitionSpec("core"),) * (n_params + n_outs),
            out_specs=(PartitionSpec("core"),) * n_outs,
            check_rep=False,
        ),
        donate_argnums=donate,
        keep_unused=True,
    )

    # The donated output-seed buffers never leave the device: a jitted
    # sharded zeros-maker replaces an 11MB host->device upload per call.
    import jax.numpy as jnp
    from jax.sharding import NamedSharding

    zeros_sharding = tuple(
        NamedSharding(mesh, PartitionSpec("core")) for _ in zero_shapes
    )
    zeros_fn = jax.jit(
        lambda: tuple(
            jnp.zeros((NCORES * s[0], *s[1:]), dt) for s, dt in zero_shapes
        ),
        out_shardings=zeros_sharding,
    )

    from concurrent.futures import ThreadPoolExecutor

    in_sharding = NamedSharding(mesh, PartitionSpec("core"))

    def run(in_maps):
        t = [time.perf_counter()]
        per_core = [
            [np.ascontiguousarray(m[name]) for m in in_maps] for name in in_names
        ]
        zeros = zeros_fn()
        _mark(t, "  run.concat")

        def _up(job):
            i, c = job
            return i, c, jax.device_put(per_core[i][c], devices[c])

        singles = [[None] * NCORES for _ in range(n_params)]
        jobs = [(i, c) for i in range(n_params) for c in range(NCORES)]
        with ThreadPoolExecutor(max_workers=NCORES) as ex:
            for i, c, arr in ex.map(_up, jobs):
                singles[i][c] = arr
        dev_in = [
            jax.make_array_from_single_device_arrays(
                (NCORES * per_core[i][0].shape[0], *per_core[i][0].shape[1:]),
                in_sharding,
                singles[i],
            )
            for i in range(n_params)
        ]
        _mark(t, "  run.upload")
        out_arrs = sharded(*dev_in, *zeros)
        _mark(t, "  run.dispatch")
        fjobs = []
        for i, a in enumerate(out_arrs):
            rows = out_avals[i].shape[0]
            for sh in a.addressable_shards:
                c = sh.index[0].start // rows if sh.index[0].start else 0
                fjobs.append((i, c, sh.data))

        def _fetch(job):
            i, c, data = job
            t0 = time.perf_counter()
            arr = np.asarray(data)
            return i, c, arr, time.perf_counter() - t0

        res = [
            np.empty((NCORES, *out_avals[i].shape), out_avals[i].dtype)
            for i in range(n_outs)
        ]
        times = []
        with ThreadPoolExecutor(max_workers=NCORES) as ex:
            for i, c, arr, dt in ex.map(_fetch, fjobs):
                res[i][c] = arr.reshape(out_avals[i].shape)
                times.append(round(dt * 1e3))
        if _TIMING:
            print(f"[phase]   fetch per-shard ms: {times}", file=sys.stderr)
        _mark(t, "  run.exec+fetch")
        return res

    return run


def _unfold(x1):
    """x1: [C_in, H, W] -> U [10000, 1600] (kept for test.py's oracle)."""
    from numpy.lib.stride_tricks import sliding_window_view

    xp2 = np.pad(x1, ((0, 0), (4, 4), (4, 4)))
    sw = sliding_window_view(xp2, (DS, DS), axis=(1, 2))
    return np.ascontiguousarray(
        sw.transpose(1, 2, 0, 3, 4).reshape(100 * 100, K), dtype=np.float32
    )


def _prep_in_maps(x, y):
    import ml_dtypes

    bf16 = ml_dtypes.bfloat16
    in_maps = []
    for s in range(N):
        xs = x[s, 0]
        ys = y[s, :, 0]
        yT = ys.transpose(1, 0, 2)                              # [96, 4, 96]
        xpad = np.zeros((C_IN, HP, HP), np.float32)
        xpad[:, DS - 1:DS - 1 + H, DS - 1:DS - 1 + W] = xs
        xpfT = xpad.transpose(1, 0, 2)                          # [104, 64, 104]
        for half in range(2):
            packed = np.zeros((HP, NBLK, WV), np.float32)
            packed[:, :C_IN, :] = xpfT[:, :, WH * half:WH * half + WV]
            packed[:H, C_IN:, :WH] = yT[:, :, WH * half:WH * (half + 1)]
            in_maps.append({"inp": packed.reshape(HP, COLS).astype(bf16)})
    return in_maps


def kernel(x, d, y, alpha, reg):
    from numpy.lib.stride_tricks import sliding_window_view

    t = [time.perf_counter()]
    x = np.asarray(x, dtype=np.float32)
    d = np.asarray(d, dtype=np.float32)
    y = np.asarray(y, dtype=np.float32)
    alpha = np.asarray(alpha, dtype=np.float32)
    reg = np.asarray(reg, dtype=np.float32)

    if "run" not in _CACHED:
        _CACHED["run"] = _build_runner()
    run = _CACHED["run"]
    _mark(t, "build")

    in_maps = _prep_in_maps(x, y)
    _mark(t, "prep")

    res1 = run(in_maps)[0]               # [8, 64, 2880 + 1600] bf16
    _mark(t, "spmd_run")

    a = alpha.reshape(N) * H * W * float(reg[0]) / (DS * DS * C_IN)
    Qs = np.empty((N, K, K), np.float32)
    Ps = np.empty((N, K, C_OUT), np.float32)
    out = np.empty((N, C_OUT, C_IN, DS, DS), dtype=np.float32)

    def _gather(s):
        o = np.asarray(res1[2 * s], np.float32) + np.asarray(res1[2 * s + 1], np.float32)
        o1 = o[:, :UF * 2 * GW]
        o2 = o[:C_OUT, UF * 2 * GW:]
        # o1 columns are (u<5, ihalf, i_local, v) -> [j, i, u, v]
        cl = np.ascontiguousarray(
            o1.reshape(C_IN, UF, 2, 32, NU).transpose(0, 2, 3, 1, 4)
        ).reshape(C_IN, C_IN, UF, NU)
        # corr[j,i,u,v]; u>=5 from symmetry corr[j,i,u,v] = corr[i,j,8-u,8-v]
        corr = np.empty((C_IN, C_IN, NU, NU), np.float32)
        corr[:, :, :UF, :] = cl
        corr[:, :, UF:, :] = np.flip(
            cl.transpose(1, 0, 2, 3)[:, :, :NU - UF, :], axis=(2, 3)
        )

        # Q[(j,kh,kw),(i,ph,pw)] = corr[j, i, ph-kh+4, pw-kw+4]
        swv = sliding_window_view(corr, (DS, DS), axis=(2, 3))   # [j,i,a,b,ph,pw]
        Q4 = swv[:, :, ::-1, ::-1, :, :].transpose(0, 2, 3, 1, 4, 5)
        Q = Qs[s].reshape(C_IN, DS, DS, C_IN, DS, DS)
        np.copyto(Q, Q4)
        Qs[s].flat[::K + 1] += a[s]

        # o2 columns are (u-2, i, v-2) for u,v in 2..6 -> P[(i,ph,pw), co]
        p2u = o2.reshape(C_OUT, DS, C_IN, DS)
        np.copyto(Ps[s].reshape(C_IN, DS, DS, C_OUT), p2u.transpose(2, 1, 3, 0))
        Ps[s] += a[s] * d[s].transpose(1, 2, 3, 0).reshape(K, C_OUT)

        # Conjugate gradient: Q is SPD with kappa ~ 6, so 12 iterations
        # reach ~1e-5 relative residual -- far below the bf16 noise floor.
        Q = Qs[s]
        X = np.zeros((K, C_OUT), np.float32)
        R = Ps[s].copy()
        Pc = R.copy()
        rs = np.einsum("kc,kc->c", R, R)
        for _ in range(12):
            QP = Q @ Pc
            al = rs / np.einsum("kc,kc->c", Pc, QP)
            X += Pc * al
            R -= QP * al
            rs_new = np.einsum("kc,kc->c", R, R)
            Pc = R + (rs_new / rs) * Pc
            rs = rs_new
        out[s] = X.reshape(C_IN, DS, DS, C_OUT).transpose(3, 0, 1, 2)

    from concurrent.futures import ThreadPoolExecutor

    with ThreadPoolExecutor(max_workers=N) as ex:
        list(ex.map(_gather, range(N)))
    _mark(t, "host_post")
    return np.ascontiguousarray(out)


# revision 46
# speedup vs baseline: 1.8002x; 1.0834x over previous
"""Bass/Trainium2 kernel for nn_DCDicl (DSBlock forward).

Algorithm: instead of the O(K^2 * R) unfold-Gram (baseline), compute the
all-pairs shift correlation corr[j,i,u,v] = sum_{h,w} x[j,h,w] *
xpad[i,h+u-4,w+v-4] (8x fewer FLOPs — the Gram is a Toeplitz gather of
corr), plus the U^T y rows for P folded into the same matmuls.

Device (8 cores = 4 samples x 2 w-halves, bf16 in / fp32 psum):
  out[m, (u,i,v)] = sum_{h, w in half} XY[m,h,w] * xpad[i, h+u, w+v]
with contraction over h (96 partitions) and PSUM accumulation over w.
Host: sum halves, gather Q via a sliding-window view, fp32 Cholesky solve.
"""

import sys
import time

import numpy as np

if "/opt/trn_rl_repo" not in sys.path:
    sys.path.append("/opt/trn_rl_repo")

N, C_IN, C_OUT, H, W, DS = 4, 64, 4, 96, 96, 5
K = C_IN * DS * DS          # 1600
NU = 2 * DS - 1             # 9 shifts per axis
M = C_IN + C_OUT            # 68 lhs rows (64 x-channels + 4 y-channels)
WH = W // 2                 # 48 w-columns per core (contraction half)
WV = WH + NU - 1            # 56 w-columns of padded image needed per core
HP = H + 2 * (DS - 1)       # 104 padded rows
NBLK = C_IN + C_OUT         # 68 56-wide column blocks (64 padded-x + 4 y)
COLS = NBLK * WV            # 3808 columns of the packed input
NUK = 7                     # computed u-shifts 0..6 (7,8 come from symmetry)
UF = 5                      # u-shifts computed for the x-x correlation
GW = 32 * NU                # 288 columns per x-corr accumulation group
GY = C_IN * DS              # 320 columns per y-corr accumulation group
NCORES = 8

_CACHED = {}
_TIMING = True


def _mark(t, name):
    if _TIMING:
        now = time.perf_counter()
        print(f"[phase] {name}: {now - t[0]:.3f}s", file=sys.stderr)
        t[0] = now


def _build_nc():
    import concourse.bass as bass
    import concourse.mybir as mybir
    from concourse.tile import TileContext

    nc = bass.Bass()
    inp = nc.dram_tensor("inp", [HP, COLS], mybir.dt.bfloat16, kind="ExternalInput")
    out1 = nc.dram_tensor(
        "o1", [C_IN, UF * 2 * GW + DS * GY], mybir.dt.bfloat16, kind="ExternalOutput"
    )

    with TileContext(nc) as tc:
        with (
            tc.tile_pool(name="inp_p", bufs=1) as inp_p,
            tc.tile_pool(name="ps_p", bufs=6, space="PSUM") as ps_p,
            tc.tile_pool(name="py_p", bufs=2, space="PSUM") as py_p,
            tc.tile_pool(name="st_p", bufs=1) as st_p,
        ):
            # One DMA materializes all 7 u-shifted replicas via an
            # overlapping sliding-window source AP: all_t[h, u, b, w] =
            # inp[h+u, b, w].  A single DMA completion sem keeps every
            # matmul at <=1 attached sync wait (the HW limit).  The
            # unpadded x itself (the matmul lhsT) is the interior of the
            # u=4 replica, so x is shipped only once.
            all_t = inp_p.tile([H, NUK, NBLK, WV], mybir.dt.bfloat16)
            src = inp[:, :]
            v = src.ap
            v.clear()
            v.extend([(COLS, H), (COLS, NUK), (WV, NBLK), (1, WV)])
            nc.sync.dma_start(out=all_t[:, :, :, :], in_=src)

            stage1 = st_p.tile([C_IN, UF * 2 * GW + DS * GY], mybir.dt.bfloat16)
            nc.vector.memset(stage1[:, :], 0)
            # x-x correlation: corr[j, i, u, v], u in 0..4 (rest by symmetry)
            for u in range(UF):
                for ihalf in range(2):
                    ps = ps_p.tile([C_IN, GW], mybir.dt.float32)
                    for wl in range(WH):
                        nc.tensor.matmul(
                            ps[:, :],
                            all_t[:, 4, 0:C_IN, wl + 4],
                            all_t[:, u, ihalf * 32:(ihalf + 1) * 32, wl:wl + NU],
                            start=(wl == 0),
                            stop=(wl == WH - 1),
                        )
                    col = (u * 2 + ihalf) * GW
                    nc.vector.tensor_copy(stage1[:, col:col + GW], ps[:, :])
            # y-x correlation: p2[co, i, u, v], u in 2..6, v in 2..6
            for ui in range(DS):
                psy = py_p.tile([C_OUT, GY], mybir.dt.float32)
                for wl in range(WH):
                    nc.tensor.matmul(
                        psy[:, :],
                        all_t[:, 0, C_IN:NBLK, wl],
                        all_t[:, ui + 2, 0:C_IN, wl + 2:wl + 2 + DS],
                        start=(wl == 0),
                        stop=(wl == WH - 1),
                    )
                col = UF * 2 * GW + ui * GY
                nc.vector.tensor_copy(stage1[:C_OUT, col:col + GY], psy[:, :])
            nc.sync.dma_start(out=out1[:, :], in_=stage1[:, :])

    _split_multiwait(nc)
    return nc


def _split_multiwait(nc):
    """Walrus rejects instructions carrying more than one attached sync wait.

    For any instruction with N>1 waits, hoist N-1 of them onto same-engine
    NoOps inserted immediately before it.
    """
    import concourse.mybir as mybir

    for fobj in nc.m.functions:
        for blk in fobj.blocks:
            insts = blk.instructions
            k = 0
            while k < len(insts):
                inst = insts[k]
                si = inst.sync_info
                if si is not None and len(si.on_wait) > 1:
                    waits = list(si.on_wait)
                    for j, w in enumerate(waits[:-1]):
                        d = mybir.InstNoOp(
                            name=f"{inst.name}_w{j}",
                            engine=inst.engine,
                            bass_nofuse=True,
                            sync_info=mybir.SyncInfo(on_wait=[w], on_update=[]),
                        )
                        nc.register_instruction(d)
                        insts.insert(k, d)
                        k += 1
                    inst.sync_info = mybir.SyncInfo(
                        on_wait=[waits[-1]], on_update=list(si.on_update)
                    )
                k += 1


def _build_runner():
    """Build the bass module once and return a cached jitted SPMD callable.

    Mirrors bass2jax.run_bass_via_pjrt's multi-core path, but the jitted
    shard_map is constructed a single time so later calls skip
    trace/lower/compile entirely.
    """
    import jax
    import concourse.mybir as mybir
    from concourse.bass2jax import (
        _bass_exec_p,
        install_neuronx_cc_hook,
        partition_id_tensor,
    )
    from jax.experimental.shard_map import shard_map
    from jax.sharding import Mesh, PartitionSpec

    nc = _build_nc()
    if not nc.is_finalized():
        nc.finalize()
    install_neuronx_cc_hook()
    assert nc.dbg_addr is None
    partition_name = (
        nc.partition_id_tensor.name if nc.partition_id_tensor is not None else None
    )

    in_names, out_names, out_avals, zero_shapes = [], [], [], []
    for alloc in nc.m.functions[0].allocations:
        if not isinstance(alloc, mybir.MemoryLocationSet):
            continue
        name = alloc.memorylocations[0].name
        if alloc.kind == "ExternalInput":
            if name != partition_name:
                in_names.append(name)
        elif alloc.kind == "ExternalOutput":
            shape = tuple(alloc.tensor_shape)
            dtype = mybir.dt.np(alloc.dtype)
            out_names.append(name)
            out_avals.append(jax.core.ShapedArray(shape, dtype))
            zero_shapes.append((shape, dtype))
    n_params = len(in_names)
    n_outs = len(out_avals)
    all_names = in_names + out_names
    if partition_name is not None:
        all_names = all_names + [partition_name]

    def _body(*args):
        operands = list(args)
        if partition_name is not None:
            operands.append(partition_id_tensor())
        outs = _bass_exec_p.bind(
            *operands,
            out_avals=tuple(out_avals),
            in_names=tuple(all_names),
            out_names=tuple(out_names),
            lowering_input_output_aliases=(),
            sim_require_finite=True,
            sim_require_nnan=True,
            nc=nc,
        )
        return tuple(outs)

    devices = jax.devices()[:NCORES]
    mesh = Mesh(np.asarray(devices), ("core",))
    donate = tuple(range(n_params, n_params + n_outs))
    sharded = jax.jit(
        shard_map(
            _body,
            mesh=mesh,
            in_specs=(PartitionSpec("core"),) * (n_params + n_outs),
            out_specs=(PartitionSpec("core"),) * n_outs,
            check_rep=False,
        ),
        donate_argnums=donate,
        keep_unused=True,
    )

    # The donated output-seed buffers never leave the device: a jitted
    # sharded zeros-maker replaces an 11MB host->device upload per call.
    import jax.numpy as jnp
    from jax.sharding import NamedSharding

    zeros_sharding = tuple(
        NamedSharding(mesh, PartitionSpec("core")) for _ in zero_shapes
    )
    zeros_fn = jax.jit(
        lambda: tuple(
            jnp.zeros((NCORES * s[0], *s[1:]), dt) for s, dt in zero_shapes
        ),
        out_shardings=zeros_sharding,
    )

    from concurrent.futures import ThreadPoolExecutor

    in_sharding = NamedSharding(mesh, PartitionSpec("core"))

    def run(in_maps):
        t = [time.perf_counter()]
        per_core = [
            [np.ascontiguousarray(m[name]) for m in in_maps] for name in in_names
        ]
        zeros = zeros_fn()
        _mark(t, "  run.concat")

        def _up(job):
            i, c = job
            return i, c, jax.device_put(per_core[i][c], devices[c])

        singles = [[None] * NCORES for _ in range(n_params)]
        jobs = [(i, c) for i in range(n_params) for c in range(NCORES)]
        with ThreadPoolExecutor(max_workers=NCORES) as ex:
            for i, c, arr in ex.map(_up, jobs):
                singles[i][c] = arr
        dev_in = [
            jax.make_array_from_single_device_arrays(
                (NCORES * per_core[i][0].shape[0], *per_core[i][0].shape[1:]),
                in_sharding,
                singles[i],
            )
            for i in range(n_params)
        ]
        _mark(t, "  run.upload")
        out_arrs = sharded(*dev_in, *zeros)
        # Queue the D2H copies now so each shard streams back as soon as
        # its core finishes, instead of after a global barrier.
        shards = {}
        for i, a in enumerate(out_arrs):
            rows = out_avals[i].shape[0]
            for sh in a.addressable_shards:
                c = sh.index[0].start // rows if sh.index[0].start else 0
                try:
                    sh.data.copy_to_host_async()
                except Exception:
                    pass
                shards[(i, c)] = sh.data
        _mark(t, "  run.dispatch")
        return shards

    return run


def _unfold(x1):
    """x1: [C_in, H, W] -> U [10000, 1600] (kept for test.py's oracle)."""
    from numpy.lib.stride_tricks import sliding_window_view

    xp2 = np.pad(x1, ((0, 0), (4, 4), (4, 4)))
    sw = sliding_window_view(xp2, (DS, DS), axis=(1, 2))
    return np.ascontiguousarray(
        sw.transpose(1, 2, 0, 3, 4).reshape(100 * 100, K), dtype=np.float32
    )


def _prep_in_maps(x, y):
    import ml_dtypes

    bf16 = ml_dtypes.bfloat16
    in_maps = []
    for s in range(N):
        xs = x[s, 0]
        ys = y[s, :, 0]
        yT = ys.transpose(1, 0, 2)                              # [96, 4, 96]
        xpad = np.zeros((C_IN, HP, HP), np.float32)
        xpad[:, DS - 1:DS - 1 + H, DS - 1:DS - 1 + W] = xs
        xpfT = xpad.transpose(1, 0, 2)                          # [104, 64, 104]
        for half in range(2):
            packed = np.zeros((HP, NBLK, WV), np.float32)
            packed[:, :C_IN, :] = xpfT[:, :, WH * half:WH * half + WV]
            packed[:H, C_IN:, :WH] = yT[:, :, WH * half:WH * (half + 1)]
            in_maps.append({"inp": packed.reshape(HP, COLS).astype(bf16)})
    return in_maps


def kernel(x, d, y, alpha, reg):
    from numpy.lib.stride_tricks import sliding_window_view

    t = [time.perf_counter()]
    x = np.asarray(x, dtype=np.float32)
    d = np.asarray(d, dtype=np.float32)
    y = np.asarray(y, dtype=np.float32)
    alpha = np.asarray(alpha, dtype=np.float32)
    reg = np.asarray(reg, dtype=np.float32)

    if "run" not in _CACHED:
        _CACHED["run"] = _build_runner()
    run = _CACHED["run"]
    _mark(t, "build")

    in_maps = _prep_in_maps(x, y)
    _mark(t, "prep")

    shards = run(in_maps)                # {(0, core): [64, 2880+1600] bf16}
    _mark(t, "spmd_run")

    a = alpha.reshape(N) * H * W * float(reg[0]) / (DS * DS * C_IN)
    Qs = np.empty((N, K, K), np.float32)
    Ps = np.empty((N, K, C_OUT), np.float32)
    out = np.empty((N, C_OUT, C_IN, DS, DS), dtype=np.float32)

    def _gather(s):
        o = np.asarray(shards[(0, 2 * s)], np.float32) + np.asarray(
            shards[(0, 2 * s + 1)], np.float32
        )
        o1 = o[:, :UF * 2 * GW]
        o2 = o[:C_OUT, UF * 2 * GW:]
        # o1 columns are (u<5, ihalf, i_local, v) -> [j, i, u, v]
        cl = np.ascontiguousarray(
            o1.reshape(C_IN, UF, 2, 32, NU).transpose(0, 2, 3, 1, 4)
        ).reshape(C_IN, C_IN, UF, NU)
        # corr[j,i,u,v]; u>=5 from symmetry corr[j,i,u,v] = corr[i,j,8-u,8-v]
        corr = np.empty((C_IN, C_IN, NU, NU), np.float32)
        corr[:, :, :UF, :] = cl
        corr[:, :, UF:, :] = np.flip(
            cl.transpose(1, 0, 2, 3)[:, :, :NU - UF, :], axis=(2, 3)
        )

        # Q[(j,kh,kw),(i,ph,pw)] = corr[j, i, ph-kh+4, pw-kw+4]
        swv = sliding_window_view(corr, (DS, DS), axis=(2, 3))   # [j,i,a,b,ph,pw]
        Q4 = swv[:, :, ::-1, ::-1, :, :].transpose(0, 2, 3, 1, 4, 5)
        Q = Qs[s].reshape(C_IN, DS, DS, C_IN, DS, DS)
        np.copyto(Q, Q4)
        Qs[s].flat[::K + 1] += a[s]

        # o2 columns are (u-2, i, v-2) for u,v in 2..6 -> P[(i,ph,pw), co]
        p2u = o2.reshape(C_OUT, DS, C_IN, DS)
        np.copyto(Ps[s].reshape(C_IN, DS, DS, C_OUT), p2u.transpose(2, 1, 3, 0))
        Ps[s] += a[s] * d[s].transpose(1, 2, 3, 0).reshape(K, C_OUT)

        # Conjugate gradient: Q is SPD with kappa ~ 6, so 12 iterations
        # reach ~1e-5 relative residual -- far below the bf16 noise floor.
        Q = Qs[s]
        X = np.zeros((K, C_OUT), np.float32)
        R = Ps[s].copy()
        Pc = R.copy()
        rs = np.einsum("kc,kc->c", R, R)
        for _ in range(12):
            QP = Q @ Pc
            al = rs / np.einsum("kc,kc->c", Pc, QP)
            X += Pc * al
            R -= QP * al
            rs_new = np.einsum("kc,kc->c", R, R)
            Pc = R + (rs_new / rs) * Pc
            rs = rs_new
        out[s] = X.reshape(C_IN, DS, DS, C_OUT).transpose(3, 0, 1, 2)

    from concurrent.futures import ThreadPoolExecutor

    with ThreadPoolExecutor(max_workers=N) as ex:
        list(ex.map(_gather, range(N)))
    _mark(t, "host_post")
    return np.ascontiguousarray(out)


# revision 49
# speedup vs baseline: 1.9685x; 1.0935x over previous
"""Bass/Trainium2 kernel for nn_DCDicl (DSBlock forward).

Algorithm: instead of the O(K^2 * R) unfold-Gram (baseline), compute the
all-pairs shift correlation corr[j,i,u,v] = sum_{h,w} x[j,h,w] *
xpad[i,h+u-4,w+v-4] (8x fewer FLOPs — the Gram is a Toeplitz gather of
corr), plus the U^T y rows for P folded into the same matmuls.

Device (8 cores = 4 samples x 2 w-halves, bf16 in / fp32 psum):
  out[m, (u,i,v)] = sum_{h, w in half} XY[m,h,w] * xpad[i, h+u, w+v]
with contraction over h (96 partitions) and PSUM accumulation over w.
Host: sum halves, gather Q via a sliding-window view, fp32 Cholesky solve.
"""

import sys
import time

import numpy as np

if "/opt/trn_rl_repo" not in sys.path:
    sys.path.append("/opt/trn_rl_repo")

N, C_IN, C_OUT, H, W, DS = 4, 64, 4, 96, 96, 5
K = C_IN * DS * DS          # 1600
NU = 2 * DS - 1             # 9 shifts per axis
M = C_IN + C_OUT            # 68 lhs rows (64 x-channels + 4 y-channels)
WH = W // 2                 # 48 w-columns per core (contraction half)
WV = WH + NU - 1            # 56 w-columns of padded image needed per core
HP = H + 2 * (DS - 1)       # 104 padded rows
NBLK = C_IN + C_OUT         # 68 56-wide column blocks (64 padded-x + 4 y)
COLS = NBLK * WV            # 3808 columns of the packed input
NUK = 7                     # computed u-shifts 0..6 (7,8 come from symmetry)
UF = 5                      # u-shifts computed for the x-x correlation
GW = 32 * NU                # 288 columns per x-corr accumulation group
GY = C_IN * DS              # 320 columns per y-corr accumulation group
NCORES = 8

_CACHED = {}
_TIMING = True


def _mark(t, name):
    if _TIMING:
        now = time.perf_counter()
        print(f"[phase] {name}: {now - t[0]:.3f}s", file=sys.stderr)
        t[0] = now


def _build_nc():
    import concourse.bass as bass
    import concourse.mybir as mybir
    from concourse.tile import TileContext

    nc = bass.Bass()
    inp = nc.dram_tensor("inp", [HP, COLS], mybir.dt.bfloat16, kind="ExternalInput")
    out1 = nc.dram_tensor(
        "o1", [C_IN, UF * 2 * GW + DS * GY], mybir.dt.bfloat16, kind="ExternalOutput"
    )

    with TileContext(nc) as tc:
        with (
            tc.tile_pool(name="inp_p", bufs=1) as inp_p,
            tc.tile_pool(name="ps_p", bufs=6, space="PSUM") as ps_p,
            tc.tile_pool(name="py_p", bufs=2, space="PSUM") as py_p,
            tc.tile_pool(name="st_p", bufs=1) as st_p,
        ):
            # One DMA materializes all 7 u-shifted replicas via an
            # overlapping sliding-window source AP: all_t[h, u, b, w] =
            # inp[h+u, b, w].  A single DMA completion sem keeps every
            # matmul at <=1 attached sync wait (the HW limit).  The
            # unpadded x itself (the matmul lhsT) is the interior of the
            # u=4 replica, so x is shipped only once.
            all_t = inp_p.tile([H, NUK, NBLK, WV], mybir.dt.bfloat16)
            src = inp[:, :]
            v = src.ap
            v.clear()
            v.extend([(COLS, H), (COLS, NUK), (WV, NBLK), (1, WV)])
            nc.sync.dma_start(out=all_t[:, :, :, :], in_=src)

            stage1 = st_p.tile([C_IN, UF * 2 * GW + DS * GY], mybir.dt.bfloat16)
            nc.vector.memset(stage1[:, :], 0)
            # x-x correlation: corr[j, i, u, v], u in 0..4 (rest by symmetry)
            for u in range(UF):
                for ihalf in range(2):
                    ps = ps_p.tile([C_IN, GW], mybir.dt.float32)
                    for wl in range(WH):
                        nc.tensor.matmul(
                            ps[:, :],
                            all_t[:, 4, 0:C_IN, wl + 4],
                            all_t[:, u, ihalf * 32:(ihalf + 1) * 32, wl:wl + NU],
                            start=(wl == 0),
                            stop=(wl == WH - 1),
                        )
                    col = (u * 2 + ihalf) * GW
                    nc.vector.tensor_copy(stage1[:, col:col + GW], ps[:, :])
            # y-x correlation: p2[co, i, u, v], u in 2..6, v in 2..6
            for ui in range(DS):
                psy = py_p.tile([C_OUT, GY], mybir.dt.float32)
                for wl in range(WH):
                    nc.tensor.matmul(
                        psy[:, :],
                        all_t[:, 0, C_IN:NBLK, wl],
                        all_t[:, ui + 2, 0:C_IN, wl + 2:wl + 2 + DS],
                        start=(wl == 0),
                        stop=(wl == WH - 1),
                    )
                col = UF * 2 * GW + ui * GY
                nc.vector.tensor_copy(stage1[:C_OUT, col:col + GY], psy[:, :])
            nc.sync.dma_start(out=out1[:, :], in_=stage1[:, :])

    _split_multiwait(nc)
    return nc


def _split_multiwait(nc):
    """Walrus rejects instructions carrying more than one attached sync wait.

    For any instruction with N>1 waits, hoist N-1 of them onto same-engine
    NoOps inserted immediately before it.
    """
    import concourse.mybir as mybir

    for fobj in nc.m.functions:
        for blk in fobj.blocks:
            insts = blk.instructions
            k = 0
            while k < len(insts):
                inst = insts[k]
                si = inst.sync_info
                if si is not None and len(si.on_wait) > 1:
                    waits = list(si.on_wait)
                    for j, w in enumerate(waits[:-1]):
                        d = mybir.InstNoOp(
                            name=f"{inst.name}_w{j}",
                            engine=inst.engine,
                            bass_nofuse=True,
                            sync_info=mybir.SyncInfo(on_wait=[w], on_update=[]),
                        )
                        nc.register_instruction(d)
                        insts.insert(k, d)
                        k += 1
                    inst.sync_info = mybir.SyncInfo(
                        on_wait=[waits[-1]], on_update=list(si.on_update)
                    )
                k += 1


def _build_runner():
    """Build the bass module once and return a cached jitted SPMD callable.

    Mirrors bass2jax.run_bass_via_pjrt's multi-core path, but the jitted
    shard_map is constructed a single time so later calls skip
    trace/lower/compile entirely.
    """
    import jax
    import concourse.mybir as mybir
    from concourse.bass2jax import (
        _bass_exec_p,
        install_neuronx_cc_hook,
        partition_id_tensor,
    )
    from jax.experimental.shard_map import shard_map
    from jax.sharding import Mesh, PartitionSpec

    nc = _build_nc()
    if not nc.is_finalized():
        nc.finalize()
    install_neuronx_cc_hook()
    assert nc.dbg_addr is None
    partition_name = (
        nc.partition_id_tensor.name if nc.partition_id_tensor is not None else None
    )

    in_names, out_names, out_avals, zero_shapes = [], [], [], []
    for alloc in nc.m.functions[0].allocations:
        if not isinstance(alloc, mybir.MemoryLocationSet):
            continue
        name = alloc.memorylocations[0].name
        if alloc.kind == "ExternalInput":
            if name != partition_name:
                in_names.append(name)
        elif alloc.kind == "ExternalOutput":
            shape = tuple(alloc.tensor_shape)
            dtype = mybir.dt.np(alloc.dtype)
            out_names.append(name)
            out_avals.append(jax.core.ShapedArray(shape, dtype))
            zero_shapes.append((shape, dtype))
    n_params = len(in_names)
    n_outs = len(out_avals)
    all_names = in_names + out_names
    if partition_name is not None:
        all_names = all_names + [partition_name]

    def _body(*args):
        operands = list(args)
        if partition_name is not None:
            operands.append(partition_id_tensor())
        outs = _bass_exec_p.bind(
            *operands,
            out_avals=tuple(out_avals),
            in_names=tuple(all_names),
            out_names=tuple(out_names),
            lowering_input_output_aliases=(),
            sim_require_finite=True,
            sim_require_nnan=True,
            nc=nc,
        )
        return tuple(outs)

    devices = jax.devices()[:NCORES]
    mesh = Mesh(np.asarray(devices), ("core",))
    sharded = jax.jit(
        shard_map(
            _body,
            mesh=mesh,
            in_specs=(PartitionSpec("core"),) * (n_params + n_outs),
            out_specs=(PartitionSpec("core"),) * n_outs,
            check_rep=False,
        ),
        keep_unused=True,
    )

    # The output-seed operands are never read (the kernel writes every
    # output element), so persistent device-resident zeros are reused
    # across calls instead of donating fresh buffers each time.
    import jax.numpy as jnp
    from jax.sharding import NamedSharding

    zeros_sharding = tuple(
        NamedSharding(mesh, PartitionSpec("core")) for _ in zero_shapes
    )
    zeros_const = jax.jit(
        lambda: tuple(
            jnp.zeros((NCORES * s[0], *s[1:]), dt) for s, dt in zero_shapes
        ),
        out_shardings=zeros_sharding,
    )()

    from concurrent.futures import ThreadPoolExecutor

    in_sharding = NamedSharding(mesh, PartitionSpec("core"))

    def run(in_maps):
        t = [time.perf_counter()]
        per_core = [
            [np.ascontiguousarray(m[name]) for m in in_maps] for name in in_names
        ]
        _mark(t, "  run.concat")

        def _up(job):
            i, c = job
            return i, c, jax.device_put(per_core[i][c], devices[c])

        singles = [[None] * NCORES for _ in range(n_params)]
        jobs = [(i, c) for i in range(n_params) for c in range(NCORES)]
        with ThreadPoolExecutor(max_workers=NCORES) as ex:
            for i, c, arr in ex.map(_up, jobs):
                singles[i][c] = arr
        dev_in = [
            jax.make_array_from_single_device_arrays(
                (NCORES * per_core[i][0].shape[0], *per_core[i][0].shape[1:]),
                in_sharding,
                singles[i],
            )
            for i in range(n_params)
        ]
        _mark(t, "  run.upload")
        out_arrs = sharded(*dev_in, *zeros_const)
        # Queue the D2H copies now so each shard streams back as soon as
        # its core finishes, instead of after a global barrier.
        shards = {}
        for i, a in enumerate(out_arrs):
            rows = out_avals[i].shape[0]
            for sh in a.addressable_shards:
                c = sh.index[0].start // rows if sh.index[0].start else 0
                try:
                    sh.data.copy_to_host_async()
                except Exception:
                    pass
                shards[(i, c)] = sh.data
        _mark(t, "  run.dispatch")
        return shards

    return run


def _unfold(x1):
    """x1: [C_in, H, W] -> U [10000, 1600] (kept for test.py's oracle)."""
    from numpy.lib.stride_tricks import sliding_window_view

    xp2 = np.pad(x1, ((0, 0), (4, 4), (4, 4)))
    sw = sliding_window_view(xp2, (DS, DS), axis=(1, 2))
    return np.ascontiguousarray(
        sw.transpose(1, 2, 0, 3, 4).reshape(100 * 100, K), dtype=np.float32
    )


def _prep_in_maps(x, y):
    import ml_dtypes

    bf16 = ml_dtypes.bfloat16
    in_maps = []
    for s in range(N):
        xs = x[s, 0]
        ys = y[s, :, 0]
        yT = ys.transpose(1, 0, 2)                              # [96, 4, 96]
        xpad = np.zeros((C_IN, HP, HP), np.float32)
        xpad[:, DS - 1:DS - 1 + H, DS - 1:DS - 1 + W] = xs
        xpfT = xpad.transpose(1, 0, 2)                          # [104, 64, 104]
        for half in range(2):
            packed = np.zeros((HP, NBLK, WV), np.float32)
            packed[:, :C_IN, :] = xpfT[:, :, WH * half:WH * half + WV]
            packed[:H, C_IN:, :WH] = yT[:, :, WH * half:WH * (half + 1)]
            in_maps.append({"inp": packed.reshape(HP, COLS).astype(bf16)})
    return in_maps


def kernel(x, d, y, alpha, reg):
    from numpy.lib.stride_tricks import sliding_window_view

    t = [time.perf_counter()]
    x = np.asarray(x, dtype=np.float32)
    d = np.asarray(d, dtype=np.float32)
    y = np.asarray(y, dtype=np.float32)
    alpha = np.asarray(alpha, dtype=np.float32)
    reg = np.asarray(reg, dtype=np.float32)

    if "run" not in _CACHED:
        _CACHED["run"] = _build_runner()
    run = _CACHED["run"]
    _mark(t, "build")

    in_maps = _prep_in_maps(x, y)
    _mark(t, "prep")

    shards = run(in_maps)                # {(0, core): [64, 2880+1600] bf16}
    _mark(t, "spmd_run")

    a = alpha.reshape(N) * H * W * float(reg[0]) / (DS * DS * C_IN)
    Qs = np.empty((N, K, K), np.float32)
    Ps = np.empty((N, K, C_OUT), np.float32)
    out = np.empty((N, C_OUT, C_IN, DS, DS), dtype=np.float32)

    def _gather(s):
        o = np.asarray(shards[(0, 2 * s)], np.float32) + np.asarray(
            shards[(0, 2 * s + 1)], np.float32
        )
        o1 = o[:, :UF * 2 * GW]
        o2 = o[:C_OUT, UF * 2 * GW:]
        # o1 columns are (u<5, ihalf, i_local, v) -> [j, i, u, v]
        cl = np.ascontiguousarray(
            o1.reshape(C_IN, UF, 2, 32, NU).transpose(0, 2, 3, 1, 4)
        ).reshape(C_IN, C_IN, UF, NU)
        # corr[j,i,u,v]; u>=5 from symmetry corr[j,i,u,v] = corr[i,j,8-u,8-v]
        corr = np.empty((C_IN, C_IN, NU, NU), np.float32)
        corr[:, :, :UF, :] = cl
        corr[:, :, UF:, :] = np.flip(
            cl.transpose(1, 0, 2, 3)[:, :, :NU - UF, :], axis=(2, 3)
        )

        # Q[(j,kh,kw),(i,ph,pw)] = corr[j, i, ph-kh+4, pw-kw+4]
        swv = sliding_window_view(corr, (DS, DS), axis=(2, 3))   # [j,i,a,b,ph,pw]
        Q4 = swv[:, :, ::-1, ::-1, :, :].transpose(0, 2, 3, 1, 4, 5)
        Q = Qs[s].reshape(C_IN, DS, DS, C_IN, DS, DS)
        np.copyto(Q, Q4)
        Qs[s].flat[::K + 1] += a[s]

        # o2 columns are (u-2, i, v-2) for u,v in 2..6 -> P[(i,ph,pw), co]
        p2u = o2.reshape(C_OUT, DS, C_IN, DS)
        np.copyto(Ps[s].reshape(C_IN, DS, DS, C_OUT), p2u.transpose(2, 1, 3, 0))
        Ps[s] += a[s] * d[s].transpose(1, 2, 3, 0).reshape(K, C_OUT)

        # Conjugate gradient: Q is SPD with kappa ~ 6, so 12 iterations
        # reach ~1e-5 relative residual -- far below the bf16 noise floor.
        Q = Qs[s]
        X = np.zeros((K, C_OUT), np.float32)
        R = Ps[s].copy()
        Pc = R.copy()
        rs = np.einsum("kc,kc->c", R, R)
        for _ in range(12):
            QP = Q @ Pc
            al = rs / np.einsum("kc,kc->c", Pc, QP)
            X += Pc * al
            R -= QP * al
            rs_new = np.einsum("kc,kc->c", R, R)
            Pc = R + (rs_new / rs) * Pc
            rs = rs_new
        out[s] = X.reshape(C_IN, DS, DS, C_OUT).transpose(3, 0, 1, 2)

    from concurrent.futures import ThreadPoolExecutor

    with ThreadPoolExecutor(max_workers=N) as ex:
        list(ex.map(_gather, range(N)))
    _mark(t, "host_post")
    return np.ascontiguousarray(out)


# revision 51
# speedup vs baseline: 2.0686x; 1.0509x over previous
"""Bass/Trainium2 kernel for nn_DCDicl (DSBlock forward).

Algorithm: instead of the O(K^2 * R) unfold-Gram (baseline), compute the
all-pairs shift correlation corr[j,i,u,v] = sum_{h,w} x[j,h,w] *
xpad[i,h+u-4,w+v-4] (8x fewer FLOPs — the Gram is a Toeplitz gather of
corr), plus the U^T y rows for P folded into the same matmuls.

Device (8 cores = 4 samples x 2 w-halves, bf16 in / fp32 psum):
  out[m, (u,i,v)] = sum_{h, w in half} XY[m,h,w] * xpad[i, h+u, w+v]
with contraction over h (96 partitions) and PSUM accumulation over w.
Host: sum halves, gather Q via a sliding-window view, fp32 Cholesky solve.
"""

import sys
import time

import numpy as np

if "/opt/trn_rl_repo" not in sys.path:
    sys.path.append("/opt/trn_rl_repo")

N, C_IN, C_OUT, H, W, DS = 4, 64, 4, 96, 96, 5
K = C_IN * DS * DS          # 1600
NU = 2 * DS - 1             # 9 shifts per axis
M = C_IN + C_OUT            # 68 lhs rows (64 x-channels + 4 y-channels)
WH = W // 2                 # 48 w-columns per core (contraction half)
WV = WH + NU - 1            # 56 w-columns of padded image needed per core
HP = H + 2 * (DS - 1)       # 104 padded rows
NBLK = C_IN + C_OUT         # 68 56-wide column blocks (64 padded-x + 4 y)
COLS = NBLK * WV            # 3808 columns of the packed input
NUK = 7                     # computed u-shifts 0..6 (7,8 come from symmetry)
UF = 5                      # u-shifts computed for the x-x correlation
GW = 32 * NU                # 288 columns per x-corr accumulation group
GY = C_IN * DS              # 320 columns per y-corr accumulation group
NCORES = 8

_CACHED = {}
_TIMING = True


def _mark(t, name):
    if _TIMING:
        now = time.perf_counter()
        print(f"[phase] {name}: {now - t[0]:.3f}s", file=sys.stderr)
        t[0] = now


def _build_nc():
    import concourse.bass as bass
    import concourse.mybir as mybir
    from concourse.tile import TileContext

    nc = bass.Bass()
    inp = nc.dram_tensor("inp", [HP, COLS], mybir.dt.bfloat16, kind="ExternalInput")
    out1 = nc.dram_tensor(
        "o1", [C_IN, UF * 2 * GW + DS * GY], mybir.dt.bfloat16, kind="ExternalOutput"
    )

    with TileContext(nc) as tc:
        with (
            tc.tile_pool(name="inp_p", bufs=1) as inp_p,
            tc.tile_pool(name="ps_p", bufs=6, space="PSUM") as ps_p,
            tc.tile_pool(name="py_p", bufs=2, space="PSUM") as py_p,
            tc.tile_pool(name="st_p", bufs=1) as st_p,
        ):
            # One DMA materializes all 7 u-shifted replicas via an
            # overlapping sliding-window source AP: all_t[h, u, b, w] =
            # inp[h+u, b, w].  A single DMA completion sem keeps every
            # matmul at <=1 attached sync wait (the HW limit).  The
            # unpadded x itself (the matmul lhsT) is the interior of the
            # u=4 replica, so x is shipped only once.
            all_t = inp_p.tile([H, NUK, NBLK, WV], mybir.dt.bfloat16)
            src = inp[:, :]
            v = src.ap
            v.clear()
            v.extend([(COLS, H), (COLS, NUK), (WV, NBLK), (1, WV)])
            nc.sync.dma_start(out=all_t[:, :, :, :], in_=src)

            stage1 = st_p.tile([C_IN, UF * 2 * GW + DS * GY], mybir.dt.bfloat16)
            nc.vector.memset(stage1[:, :], 0)
            # x-x correlation: corr[j, i, u, v], u in 0..4 (rest by symmetry)
            for u in range(UF):
                for ihalf in range(2):
                    ps = ps_p.tile([C_IN, GW], mybir.dt.float32)
                    for wl in range(WH):
                        nc.tensor.matmul(
                            ps[:, :],
                            all_t[:, 4, 0:C_IN, wl + 4],
                            all_t[:, u, ihalf * 32:(ihalf + 1) * 32, wl:wl + NU],
                            start=(wl == 0),
                            stop=(wl == WH - 1),
                        )
                    col = (u * 2 + ihalf) * GW
                    nc.vector.tensor_copy(stage1[:, col:col + GW], ps[:, :])
            # y-x correlation: p2[co, i, u, v], u in 2..6, v in 2..6
            for ui in range(DS):
                psy = py_p.tile([C_OUT, GY], mybir.dt.float32)
                for wl in range(WH):
                    nc.tensor.matmul(
                        psy[:, :],
                        all_t[:, 0, C_IN:NBLK, wl],
                        all_t[:, ui + 2, 0:C_IN, wl + 2:wl + 2 + DS],
                        start=(wl == 0),
                        stop=(wl == WH - 1),
                    )
                col = UF * 2 * GW + ui * GY
                nc.vector.tensor_copy(stage1[:C_OUT, col:col + GY], psy[:, :])
            nc.sync.dma_start(out=out1[:, :], in_=stage1[:, :])

    _split_multiwait(nc)
    return nc


def _split_multiwait(nc):
    """Walrus rejects instructions carrying more than one attached sync wait.

    For any instruction with N>1 waits, hoist N-1 of them onto same-engine
    NoOps inserted immediately before it.
    """
    import concourse.mybir as mybir

    for fobj in nc.m.functions:
        for blk in fobj.blocks:
            insts = blk.instructions
            k = 0
            while k < len(insts):
                inst = insts[k]
                si = inst.sync_info
                if si is not None and len(si.on_wait) > 1:
                    waits = list(si.on_wait)
                    for j, w in enumerate(waits[:-1]):
                        d = mybir.InstNoOp(
                            name=f"{inst.name}_w{j}",
                            engine=inst.engine,
                            bass_nofuse=True,
                            sync_info=mybir.SyncInfo(on_wait=[w], on_update=[]),
                        )
                        nc.register_instruction(d)
                        insts.insert(k, d)
                        k += 1
                    inst.sync_info = mybir.SyncInfo(
                        on_wait=[waits[-1]], on_update=list(si.on_update)
                    )
                k += 1


def _build_runner():
    """Build the bass module once and return a cached jitted SPMD callable.

    Mirrors bass2jax.run_bass_via_pjrt's multi-core path, but the jitted
    shard_map is constructed a single time so later calls skip
    trace/lower/compile entirely.
    """
    import jax
    import concourse.mybir as mybir
    from concourse.bass2jax import (
        _bass_exec_p,
        install_neuronx_cc_hook,
        partition_id_tensor,
    )
    from jax.experimental.shard_map import shard_map
    from jax.sharding import Mesh, PartitionSpec

    nc = _build_nc()
    if not nc.is_finalized():
        nc.finalize()
    install_neuronx_cc_hook()
    assert nc.dbg_addr is None
    partition_name = (
        nc.partition_id_tensor.name if nc.partition_id_tensor is not None else None
    )

    in_names, out_names, out_avals, zero_shapes = [], [], [], []
    for alloc in nc.m.functions[0].allocations:
        if not isinstance(alloc, mybir.MemoryLocationSet):
            continue
        name = alloc.memorylocations[0].name
        if alloc.kind == "ExternalInput":
            if name != partition_name:
                in_names.append(name)
        elif alloc.kind == "ExternalOutput":
            shape = tuple(alloc.tensor_shape)
            dtype = mybir.dt.np(alloc.dtype)
            out_names.append(name)
            out_avals.append(jax.core.ShapedArray(shape, dtype))
            zero_shapes.append((shape, dtype))
    n_params = len(in_names)
    n_outs = len(out_avals)
    all_names = in_names + out_names
    if partition_name is not None:
        all_names = all_names + [partition_name]

    def _body(*args):
        operands = list(args)
        if partition_name is not None:
            operands.append(partition_id_tensor())
        outs = _bass_exec_p.bind(
            *operands,
            out_avals=tuple(out_avals),
            in_names=tuple(all_names),
            out_names=tuple(out_names),
            lowering_input_output_aliases=(),
            sim_require_finite=True,
            sim_require_nnan=True,
            nc=nc,
        )
        return tuple(outs)

    devices = jax.devices()[:NCORES]
    mesh = Mesh(np.asarray(devices), ("core",))
    sharded = jax.jit(
        shard_map(
            _body,
            mesh=mesh,
            in_specs=(PartitionSpec("core"),) * (n_params + n_outs),
            out_specs=(PartitionSpec("core"),) * n_outs,
            check_rep=False,
        ),
        keep_unused=True,
    )

    # The output-seed operands are never read (the kernel writes every
    # output element), so persistent device-resident zeros are reused
    # across calls instead of donating fresh buffers each time.
    import jax.numpy as jnp
    from jax.sharding import NamedSharding

    zeros_sharding = tuple(
        NamedSharding(mesh, PartitionSpec("core")) for _ in zero_shapes
    )
    zeros_const = jax.jit(
        lambda: tuple(
            jnp.zeros((NCORES * s[0], *s[1:]), dt) for s, dt in zero_shapes
        ),
        out_shardings=zeros_sharding,
    )()

    from concurrent.futures import ThreadPoolExecutor

    in_sharding = NamedSharding(mesh, PartitionSpec("core"))

    def run(in_maps):
        t = [time.perf_counter()]
        per_core = [
            [np.ascontiguousarray(m[name]) for m in in_maps] for name in in_names
        ]
        _mark(t, "  run.concat")

        def _up(job):
            i, c = job
            return i, c, jax.device_put(per_core[i][c], devices[c])

        singles = [[None] * NCORES for _ in range(n_params)]
        jobs = [(i, c) for i in range(n_params) for c in range(NCORES)]
        with ThreadPoolExecutor(max_workers=NCORES) as ex:
            for i, c, arr in ex.map(_up, jobs):
                singles[i][c] = arr
        dev_in = [
            jax.make_array_from_single_device_arrays(
                (NCORES * per_core[i][0].shape[0], *per_core[i][0].shape[1:]),
                in_sharding,
                singles[i],
            )
            for i in range(n_params)
        ]
        _mark(t, "  run.upload")
        out_arrs = sharded(*dev_in, *zeros_const)
        # Queue the D2H copies now so each shard streams back as soon as
        # its core finishes, instead of after a global barrier.
        shards = {}
        for i, a in enumerate(out_arrs):
            rows = out_avals[i].shape[0]
            for sh in a.addressable_shards:
                c = sh.index[0].start // rows if sh.index[0].start else 0
                try:
                    sh.data.copy_to_host_async()
                except Exception:
                    pass
                shards[(i, c)] = sh.data
        _mark(t, "  run.dispatch")
        return shards

    return run


def _unfold(x1):
    """x1: [C_in, H, W] -> U [10000, 1600] (kept for test.py's oracle)."""
    from numpy.lib.stride_tricks import sliding_window_view

    xp2 = np.pad(x1, ((0, 0), (4, 4), (4, 4)))
    sw = sliding_window_view(xp2, (DS, DS), axis=(1, 2))
    return np.ascontiguousarray(
        sw.transpose(1, 2, 0, 3, 4).reshape(100 * 100, K), dtype=np.float32
    )


def _prep_in_maps(x, y):
    import ml_dtypes

    bf16 = ml_dtypes.bfloat16
    in_maps = []
    for s in range(N):
        xs = x[s, 0]
        ys = y[s, :, 0]
        yT = ys.transpose(1, 0, 2)                              # [96, 4, 96]
        xpad = np.zeros((C_IN, HP, HP), np.float32)
        xpad[:, DS - 1:DS - 1 + H, DS - 1:DS - 1 + W] = xs
        xpfT = xpad.transpose(1, 0, 2)                          # [104, 64, 104]
        for half in range(2):
            packed = np.zeros((HP, NBLK, WV), np.float32)
            packed[:, :C_IN, :] = xpfT[:, :, WH * half:WH * half + WV]
            packed[:H, C_IN:, :WH] = yT[:, :, WH * half:WH * (half + 1)]
            in_maps.append({"inp": packed.reshape(HP, COLS).astype(bf16)})
    return in_maps


def kernel(x, d, y, alpha, reg):
    from numpy.lib.stride_tricks import sliding_window_view

    t = [time.perf_counter()]
    x = np.asarray(x, dtype=np.float32)
    d = np.asarray(d, dtype=np.float32)
    y = np.asarray(y, dtype=np.float32)
    alpha = np.asarray(alpha, dtype=np.float32)
    reg = np.asarray(reg, dtype=np.float32)

    if "run" not in _CACHED:
        _CACHED["run"] = _build_runner()
    run = _CACHED["run"]
    _mark(t, "build")

    in_maps = _prep_in_maps(x, y)
    _mark(t, "prep")

    shards = run(in_maps)                # {(0, core): [64, 2880+1600] bf16}
    _mark(t, "spmd_run")

    a = alpha.reshape(N) * H * W * float(reg[0]) / (DS * DS * C_IN)
    Qs = np.empty((N, K, K), np.float32)
    Ps = np.empty((N, K, C_OUT), np.float32)
    out = np.empty((N, C_OUT, C_IN, DS, DS), dtype=np.float32)

    tinfo = {}

    def _gather(s):
        t0 = time.perf_counter()
        o = np.asarray(shards[(0, 2 * s)], np.float32) + np.asarray(
            shards[(0, 2 * s + 1)], np.float32
        )
        t1 = time.perf_counter()
        o1 = o[:, :UF * 2 * GW]
        o2 = o[:C_OUT, UF * 2 * GW:]
        # o1 columns are (u<5, ihalf, i_local, v) -> [j, i, u, v]
        cl = np.ascontiguousarray(
            o1.reshape(C_IN, UF, 2, 32, NU).transpose(0, 2, 3, 1, 4)
        ).reshape(C_IN, C_IN, UF, NU)
        # corr[j,i,u,v]; u>=5 from symmetry corr[j,i,u,v] = corr[i,j,8-u,8-v]
        corr = np.empty((C_IN, C_IN, NU, NU), np.float32)
        corr[:, :, :UF, :] = cl
        corr[:, :, UF:, :] = np.flip(
            cl.transpose(1, 0, 2, 3)[:, :, :NU - UF, :], axis=(2, 3)
        )

        # Q[(j,kh,kw),(i,ph,pw)] = corr[j, i, ph-kh+4, pw-kw+4]
        swv = sliding_window_view(corr, (DS, DS), axis=(2, 3))   # [j,i,a,b,ph,pw]
        Q4 = swv[:, :, ::-1, ::-1, :, :].transpose(0, 2, 3, 1, 4, 5)
        Q = Qs[s].reshape(C_IN, DS, DS, C_IN, DS, DS)
        np.copyto(Q, Q4)
        Qs[s].flat[::K + 1] += a[s]

        # o2 columns are (u-2, i, v-2) for u,v in 2..6 -> P[(i,ph,pw), co]
        p2u = o2.reshape(C_OUT, DS, C_IN, DS)
        np.copyto(Ps[s].reshape(C_IN, DS, DS, C_OUT), p2u.transpose(2, 1, 3, 0))
        Ps[s] += a[s] * d[s].transpose(1, 2, 3, 0).reshape(K, C_OUT)

        # Conjugate gradient: Q is SPD with kappa ~ 6, so 12 iterations
        # reach ~1e-5 relative residual -- far below the bf16 noise floor.
        Q = Qs[s]
        X = np.zeros((K, C_OUT), np.float32)
        R = Ps[s].copy()
        Pc = R.copy()
        rs = np.einsum("kc,kc->c", R, R)
        for _ in range(12):
            QP = Q @ Pc
            al = rs / np.einsum("kc,kc->c", Pc, QP)
            X += Pc * al
            R -= QP * al
            rs_new = np.einsum("kc,kc->c", R, R)
            Pc = R + (rs_new / rs) * Pc
            rs = rs_new
        out[s] = X.reshape(C_IN, DS, DS, C_OUT).transpose(3, 0, 1, 2)
        tinfo[s] = (round((t1 - t0) * 1e3), round((time.perf_counter() - t1) * 1e3))

    from concurrent.futures import ThreadPoolExecutor

    with ThreadPoolExecutor(max_workers=N) as ex:
        list(ex.map(_gather, range(N)))
    if _TIMING:
        print(f"[phase]   per-sample (fetch_ms, comp_ms): {tinfo}", file=sys.stderr)
    _mark(t, "host_post")
    return np.ascontiguousarray(out)
